# revision 2
# baseline (speedup 1.0000x reference)
"""Multi-head attention (B=8, N=1024, C=768, H=12) on 8 TRN2 NeuronCores.

Sharding: pure data-parallel over batch - core b computes attention for x[b].
Per-core Bass/Tile kernel, bf16 compute, f32 PSUM.

v2 schedule (no dup layout):
  qkv psum for pair hp lands with head a's 64 d-rows on partitions 0-63 and
  head b's on 64-127 (natural wqk column order). S matmuls pair
  (head a, rows 0-63) with (head b, rows 64-127) adjacent in the PE queue,
  so on HW they run concurrently via row-group tiling (tile_position
  auto-derived from base_partition). One [128,1024] PSUM->SBUF copy per
  role per pair; no mirror DMAs.

  E interleaved: e_ab[128, mt, nch, ab, 512] - one exp instruction per
  (mt, nch) covers both heads; one PSUM slot [128, (ab), 512] holds the
  S pair.

  O^T per head h: 16 MMs (8 subs x 2 nch), stationary v[sub,h] [128,65]
  (ones col fused -> sums in row 64), moving E. Pair hp's phase carries
  pair hp-1's O: head a' over groups 1-3, head b' over groups 5-7, leaving
  groups 0/4 as drain windows so the freed PSUM slot is back before the
  next O allocation.

  norm: DVE recip straight from PSUM row 64 + DVE [64,1024] drain copy
  (frees the PSUM slot ~2us after O completes), Pool partition_broadcast +
  multiply from SBUF. PE/Act untouched; chains never block PE.

  PSUM budget: 4 two-bank slots - E pair-slot x2 (rotating, exp-paced),
  O accumulator x1, filler (qk chunk / v / proj partial) x1.

  tail: O(10), O(11) dense; proj partials (kt 0-3 prefilled as phase-5
  fillers) overlap the last norm chains; per-otp Act bias epilogue + DMA.
"""

import numpy as np
import ml_dtypes

B, N, C = 8, 1024, 768
H, D = 12, 64
SCALE = D ** -0.5
CT = C // 128        # 6 contraction tiles
NT = N // 128        # 8 token tiles
NCH = N // 512       # 2 n-chunks of 512
HP = H // 2          # 6 head pairs

_CACHE = {}


def _build_nc(loop_r=None):
    import concourse.bacc as bacc
    import concourse.mybir as mybir
    import concourse.tile as tile

    f32 = mybir.dt.float32
    bf16 = mybir.dt.bfloat16

    nc = bacc.Bacc("TRN2", target_bir_lowering=False, debug=False, num_devices=8)

    xT_d = nc.dram_tensor("xT", [C, N], bf16, kind="ExternalInput").ap()
    # wqk viewed [C, role(q|k), pair, 128] so one DMA descriptor per kt
    # carries q+k for a pair (or all five non-first pairs)
    wqk_d = nc.dram_tensor("wqk", [C, 2, CT, 128], bf16,
                           kind="ExternalInput").ap()
    wv_d = nc.dram_tensor("wv", [C, C], bf16, kind="ExternalInput").ap()
    wp_d = nc.dram_tensor("wp", [C, C], bf16, kind="ExternalInput").ap()
    pb_d = nc.dram_tensor("pb", [128, CT], f32, kind="ExternalInput").ap()
    out_d = nc.dram_tensor("out", [C, N], bf16, kind="ExternalOutput").ap()

    with tile.TileContext(nc) as tc:
        with (
            tc.tile_pool(name="const", bufs=1) as cpool,
            tc.tile_pool(name="E", bufs=2) as epool,
            tc.tile_pool(name="qk", bufs=4) as qkpool,
            tc.tile_pool(name="small", bufs=2) as spool,
            tc.tile_pool(name="y", bufs=4) as ypool,
            tc.tile_pool(name="ps", bufs=4, space="PSUM") as pspool,
        ):
            # ---- persistent SBUF tensors ----
            xT_sb = cpool.tile([128, CT, N], bf16)            # 12KB/part
            wqk_sb = cpool.tile([128, CT, 2, CT, 128], bf16)  # 18KB
            wv_sb = cpool.tile([128, CT, C], bf16)            # 9KB
            wp_sb = cpool.tile([128, CT, C], bf16)            # 9KB
            pb_sb = cpool.tile([128, CT], f32)
            v_sb = cpool.tile([128, NT, H, D + 1], bf16)      # 12.2KB
            on_sb = cpool.tile([128, CT, NCH, 512], bf16)     # Onorm^T, 12KB

            # input DMA, three queues by criticality: gpsimd's software
            # DGE (own issue budget) carries xT; the two HWDGE queues
            # (sync/scalar, shared issue bandwidth) carry pair-0 q+k and
            # wv first, then the remaining wqk pairs, then wp.
            for kt in range(CT):
                r = kt * 128
                nc.gpsimd.dma_start(xT_sb[:, kt, :], xT_d[r:r + 128, :])
                nc.sync.dma_start(wqk_sb[:, kt, :, 0, :],
                                  wqk_d[r:r + 128, :, 0, :])
                nc.scalar.dma_start(wv_sb[:, kt, :], wv_d[r:r + 128, :])
            for kt in range(CT):
                r = kt * 128
                nc.sync.dma_start(wqk_sb[:, kt, :, 1:CT, :],
                                  wqk_d[r:r + 128, :, 1:CT, :])
            for kt in range(CT):
                r = kt * 128
                nc.scalar.dma_start(wp_sb[:, kt, :], wp_d[r:r + 128, :])
            nc.gpsimd.dma_start(pb_sb[:], pb_d[:])
            # ones column fused into v (softmax sums emerge as O^T row 64)
            nc.vector.memset(v_sb[:, :, :, D:D + 1], 1.0)

            def body():
                qks = {}      # (hp, role) -> SBUF tile; ("ps",hp,role) -> psum
                vps = {}

                def emit_qk_chunk(hp, role, lo, hi):
                    key = ("ps", hp, role)
                    if lo == 0:
                        qks[key] = pspool.tile([128, NCH, 512], f32, tag="ps",
                                               name=f"qk{hp}r{role}")
                    ps = qks[key]
                    for kt in range(lo, hi):
                        for nch in range(NCH):
                            nc.tensor.matmul(
                                ps[:, nch, :],
                                wqk_sb[:, kt, role, hp, :],
                                xT_sb[:, kt, nch * 512:(nch + 1) * 512],
                                start=(kt == 0), stop=(kt == CT - 1),
                            )
                    if hi == CT:
                        t = qkpool.tile([128, NCH, 512], bf16, tag="qk",
                                        name=f"qk{hp}r{role}sb")
                        nc.vector.tensor_copy(t[:], ps[:, :, :])
                        qks[(hp, role)] = t
                        del qks[key]

                def emit_v_chunk(nt, lo, hi):
                    if lo == 0:
                        vps[nt] = pspool.tile([128, 2, 8, 64], f32, tag="ps",
                                              name=f"v{nt}")
                    ps = vps[nt]
                    for kt in range(lo, hi):
                        for och in range(2):
                            nc.tensor.matmul(
                                ps[:, och, 0:6, :],
                                xT_sb[:, kt, nt * 128:(nt + 1) * 128],
                                wv_sb[:, kt, och * 384:(och + 1) * 384],
                                start=(kt == 0), stop=(kt == CT - 1),
                            )
                    if hi == CT:
                        for och in range(2):
                            nc.vector.tensor_copy(
                                v_sb[:, nt, och * 6:(och + 1) * 6, 0:D],
                                ps[:, och, 0:6, :],
                            )
                        del vps[nt]

                fillers = []

                def take_fillers(k):
                    for _ in range(min(k, len(fillers))):
                        fillers.pop(0)()

                def queue_qk_chunks(hp):
                    for role in (0, 1):
                        for kt in range(CT):
                            fillers.append(
                                lambda hp=hp, role=role, kt=kt:
                                emit_qk_chunk(hp, role, kt, kt + 1))

                def queue_v_chunks(nt):
                    fillers.append(lambda: emit_v_chunk(nt, 0, 3))
                    fillers.append(lambda: emit_v_chunk(nt, 3, CT))

                def norm(h, ps_o, tail=False):
                    """O^T[0:64]/sums(row64) -> on_sb. The DVE drain copy
                    frees the PSUM slot early; Pool does broadcast+mult."""
                    hp, odd = h // 2, h % 2
                    # reciprocal_approx_fast misreads PSUM@partition-64 on
                    # HW - stage the sums row through SBUF first
                    sm = spool.tile([1, NCH, 512], f32, tag="sm")
                    if tail:
                        nc.scalar.copy(sm[:], ps_o[64:65, :, :])
                    else:
                        nc.vector.tensor_copy(sm[:], ps_o[64:65, :, :])
                    rec = spool.tile([1, NCH, 512], f32, tag="rec")
                    nc.vector.reciprocal_approx_fast(rec[:], sm[:])
                    ou = spool.tile([64, NCH, 512], bf16, tag="ou")
                    if tail:
                        nc.scalar.copy(ou[:], ps_o[0:64, :, :])
                    else:
                        nc.vector.tensor_copy(ou[:], ps_o[0:64, :, :])
                    R = spool.tile([64, NCH, 512], f32, tag="R")
                    nc.gpsimd.partition_broadcast(R[:], rec[:])
                    eng = nc.vector if tail else nc.gpsimd
                    eng.tensor_tensor(
                        on_sb[64 * odd:64 * odd + 64, hp, :, :],
                        ou[:], R[:], op=mybir.AluOpType.mult,
                    )

                # ---- startup: eager qk(pair 0) pipelined with input DMA ----
                emit_qk_chunk(0, 0, 0, CT)
                emit_qk_chunk(0, 1, 0, CT)
                emit_v_chunk(0, 0, CT)
                emit_v_chunk(1, 0, CT)

                E_prev = None
                pj = {}
                stg = {}

                def stage_proj(otp):
                    t = ypool.tile([128, NCH, 512], bf16, tag="stg",
                                   name=f"stg{otp}")
                    nc.vector.tensor_copy(t[:], pj[otp][:, :, :])
                    stg[otp] = t
                    del pj[otp]

                def proj_chunk(otp, kts, lo=0, stop_kt=CT - 1):
                    if kts[0] == lo:
                        pj[otp] = pspool.tile([128, NCH, 512], f32, tag="ps",
                                              name=f"pj{otp}k{lo}")
                    for kt in kts:
                        for nch in range(NCH):
                            nc.tensor.matmul(
                                pj[otp][:, nch, :],
                                wp_sb[:, kt, otp * 128:(otp + 1) * 128],
                                on_sb[:, kt, nch, :],
                                start=(kt == lo), stop=(kt == stop_kt),
                            )

                def proj_fin(otp):
                    # per-nch epilogue so the Act copy and output DMA of
                    # nch0 overlap the tail's remaining PE work
                    yt = ypool.tile([128, NCH, 512], bf16, tag="yt")
                    for nch in range(NCH):
                        nc.scalar.activation(
                            yt[:, nch, :], pj[otp][:, nch, :],
                            mybir.ActivationFunctionType.Identity,
                            bias=pb_sb[:, otp:otp + 1],
                        )
                        # split across software/hardware DGE queues
                        eng = nc.gpsimd if nch == 0 else nc.sync
                        eng.dma_start(
                            out_d[otp * 128:(otp + 1) * 128,
                                  nch * 512:(nch + 1) * 512],
                            yt[:, nch, :])

                for hp in range(HP):
                    q_sb, k_sb = qks[(hp, 0)], qks[(hp, 1)]
                    E_ab = epool.tile([128, NT, NCH, 2, 512], bf16, tag="E",
                                      name=f"E{hp}")
                    if hp == 0:
                        # qk(1) first: its SBUF copy gates phase 1's S MMs
                        queue_qk_chunks(1)
                        for nt in range(2, 6):
                            queue_v_chunks(nt)
                    elif hp == 1:
                        # v(6/7) first: O(pair 0) group 3 reads them mid-phase
                        for nt in range(6, 8):
                            queue_v_chunks(nt)
                        queue_qk_chunks(2)
                    elif hp < HP - 1:
                        queue_qk_chunks(hp + 1)
                    else:
                        # phase 5: proj otp-0/1 partials over ready kt 0-3,
                        # staged to SBUF via DVE (transient PSUM slot each)
                        for otp in (0, 1):
                            for kt in range(4):
                                fillers.append(
                                    lambda otp=otp, kt=kt:
                                    proj_chunk(otp, [kt], stop_kt=3))
                            fillers.append(
                                lambda otp=otp: stage_proj(otp))

                    # O-prev sub-tiles per group: groups 1-3 head a',
                    # groups 5-7 head b'; groups 0/4 are PSUM drain windows.
                    osub = {1: (0, 3), 2: (3, 6), 3: (6, 8),
                            5: (0, 3), 6: (3, 6), 7: (6, 8)}
                    ps_o = None
                    for mt in range(NT):
                        c0, c1 = mt // 4, (mt % 4) * 128
                        for nch in range(NCH):
                            ps_s = pspool.tile([128, 2, 512], f32, tag="ps",
                                               name=f"s{hp}m{mt}n{nch}")
                            nc.tensor.matmul(
                                ps_s[:, 0, :],
                                k_sb[0:64, c0, c1:c1 + 128],
                                q_sb[0:64, nch, :], start=True, stop=True,
                            )
                            nc.tensor.matmul(
                                ps_s[:, 1, :],
                                k_sb[64:128, c0, c1:c1 + 128],
                                q_sb[64:128, nch, :], start=True, stop=True,
                            )
                            nc.scalar.activation(
                                E_ab[:, mt, nch, :, :], ps_s[:, :, :],
                                mybir.ActivationFunctionType.Exp, scale=SCALE,
                            )
                        if E_prev is not None and mt in osub:
                            h_prev = 2 * (hp - 1) + (0 if mt < 4 else 1)
                            ab = h_prev % 2
                            lo, hi = osub[mt]
                            if lo == 0:
                                ps_o = pspool.tile([65, NCH, 512], f32,
                                                   tag="ps", name=f"o{h_prev}")
                            for sub in range(lo, hi):
                                for nch in range(NCH):
                                    nc.tensor.matmul(
                                        ps_o[:, nch, :],
                                        v_sb[:, sub, h_prev, :],
                                        E_prev[:, sub, nch, ab, :],
                                        start=(sub == 0), stop=(sub == NT - 1),
                                    )
                            if hi == NT:
                                norm(h_prev, ps_o)
                        # front-load pops so next pair's qk copy lands
                        # mid-phase, but keep late groups fed
                        nfill = 3 if (mt < 2 or len(fillers) > 16) else 2
                        take_fillers(nfill)
                    take_fillers(len(fillers))
                    E_prev = E_ab

                # ---- tail: O(pair 5) + projections, dense on PE ----
                ha, hb = 2 * (HP - 1), 2 * (HP - 1) + 1

                def emit_o_dense(h):
                    ps = pspool.tile([65, NCH, 512], f32, tag="ps",
                                     name=f"ot{h}")
                    ab = h % 2
                    for sub in range(NT):
                        for nch in range(NCH):
                            nc.tensor.matmul(
                                ps[:, nch, :], v_sb[:, sub, h, :],
                                E_prev[:, sub, nch, ab, :],
                                start=(sub == 0), stop=(sub == NT - 1),
                            )
                    return ps

                def proj_fin_staged(otp):
                    # (kt4+kt5 psum + bias) + staged kt0-3 partial, on DVE
                    yt = ypool.tile([128, NCH, 512], bf16, tag="yt")
                    nc.vector.scalar_tensor_tensor(
                        yt[:], pj[otp][:, :, :], pb_sb[:, otp:otp + 1],
                        stg[otp][:],
                        op0=mybir.AluOpType.add, op1=mybir.AluOpType.add,
                    )
                    for nch in range(NCH):
                        eng = nc.gpsimd if nch == 0 else nc.sync
                        eng.dma_start(
                            out_d[otp * 128:(otp + 1) * 128,
                                  nch * 512:(nch + 1) * 512],
                            yt[:, nch, :])

                ps_oa = emit_o_dense(ha)
                norm(ha, ps_oa, tail=True)
                ps_ob = emit_o_dense(hb)
                # overlap norm(11) chain with proj partial blocks
                proj_chunk(2, range(0, 5))
                norm(hb, ps_ob, tail=True)
                proj_chunk(3, range(0, 5))
                proj_chunk(0, [4, 5], lo=4)
                proj_fin_staged(0)
                proj_chunk(1, [4, 5], lo=4)
                proj_fin_staged(1)
                proj_chunk(2, [5])
                proj_fin(2)
                proj_chunk(3, [5])
                proj_fin(3)
                for otp in range(4, CT):
                    proj_chunk(otp, range(0, CT))
                    proj_fin(otp)

            if loop_r is not None:
                with tc.For_i(0, loop_r):
                    body()
            else:
                body()

    nc.compile()
    return nc


def _get_nc():
    if "nc" not in _CACHE:
        _CACHE["nc"] = _build_nc()
    return _CACHE["nc"]


def kernel(x, qkv_w, proj_w, proj_b):
    from concourse.bass_utils import run_bass_kernel_spmd

    nc = _get_nc()
    bf = ml_dtypes.bfloat16
    wqk = np.ascontiguousarray(qkv_w[:2 * C].T).astype(bf).reshape(C, 2, CT, 128)
    wv = np.ascontiguousarray(qkv_w[2 * C:].T).astype(bf)
    wp = np.ascontiguousarray(proj_w.T).astype(bf)
    pb = np.ascontiguousarray(proj_b.reshape(CT, 128).T).astype(np.float32)
    in_maps = []
    for i in range(B):
        in_maps.append({
            "xT": np.ascontiguousarray(x[i].T).astype(bf),
            "wqk": wqk, "wv": wv, "wp": wp, "pb": pb,
        })
    res = run_bass_kernel_spmd(nc, in_maps, core_ids=list(range(B)))
    out = np.stack([res.results[i]["out"].astype(np.float32).T for i in range(B)])
    return np.ascontiguousarray(out)


# revision 12
# speedup vs baseline: 1.0075x; 1.0075x over previous
"""Multi-head attention (B=8, N=1024, C=768, H=12) on 8 TRN2 NeuronCores.

Sharding: pure data-parallel over batch - core b computes attention for x[b].
Per-core Bass/Tile kernel, bf16 compute, f32 PSUM.

v2 schedule (no dup layout):
  qkv psum for pair hp lands with head a's 64 d-rows on partitions 0-63 and
  head b's on 64-127 (natural wqk column order). S matmuls pair
  (head a, rows 0-63) with (head b, rows 64-127) adjacent in the PE queue,
  so on HW they run concurrently via row-group tiling (tile_position
  auto-derived from base_partition). One [128,1024] PSUM->SBUF copy per
  role per pair; no mirror DMAs.

  E interleaved: e_ab[128, mt, nch, ab, 512] - one exp instruction per
  (mt, nch) covers both heads; one PSUM slot [128, (ab), 512] holds the
  S pair.

  O^T per head h: 16 MMs (8 subs x 2 nch), stationary v[sub,h] [128,65]
  (ones col fused -> sums in row 64), moving E. Pair hp's phase carries
  pair hp-1's O: head a' over groups 1-3, head b' over groups 5-7, leaving
  groups 0/4 as drain windows so the freed PSUM slot is back before the
  next O allocation.

  norm: DVE recip straight from PSUM row 64 + DVE [64,1024] drain copy
  (frees the PSUM slot ~2us after O completes), Pool partition_broadcast +
  multiply from SBUF. PE/Act untouched; chains never block PE.

  PSUM budget: 4 two-bank slots - E pair-slot x2 (rotating, exp-paced),
  O accumulator x1, filler (qk chunk / v / proj partial) x1.

  tail: O(10), O(11) dense; proj partials (kt 0-3 prefilled as phase-5
  fillers) overlap the last norm chains; per-otp Act bias epilogue + DMA.
"""

import numpy as np
import ml_dtypes

B, N, C = 8, 1024, 768
H, D = 12, 64
SCALE = D ** -0.5
CT = C // 128        # 6 contraction tiles
NT = N // 128        # 8 token tiles
NCH = N // 512       # 2 n-chunks of 512
HP = H // 2          # 6 head pairs

_CACHE = {}


def _build_nc(loop_r=None):
    import concourse.bacc as bacc
    import concourse.mybir as mybir
    import concourse.tile as tile

    f32 = mybir.dt.float32
    bf16 = mybir.dt.bfloat16

    nc = bacc.Bacc("TRN2", target_bir_lowering=False, debug=False, num_devices=8)

    xT_d = nc.dram_tensor("xT", [C, N], bf16, kind="ExternalInput").ap()
    # wqk viewed [C, role(q|k), pair, 128] so one DMA descriptor per kt
    # carries q+k for a pair (or all five non-first pairs)
    wqk_d = nc.dram_tensor("wqk", [C, 2, CT, 128], bf16,
                           kind="ExternalInput").ap()
    wv_d = nc.dram_tensor("wv", [C, C], bf16, kind="ExternalInput").ap()
    wp_d = nc.dram_tensor("wp", [C, C], bf16, kind="ExternalInput").ap()
    pb_d = nc.dram_tensor("pb", [128, CT], f32, kind="ExternalInput").ap()
    out_d = nc.dram_tensor("out", [C, N], bf16, kind="ExternalOutput").ap()

    with tile.TileContext(nc) as tc:
        with (
            tc.tile_pool(name="const", bufs=1) as cpool,
            tc.tile_pool(name="E", bufs=2) as epool,
            tc.tile_pool(name="qk", bufs=4) as qkpool,
            tc.tile_pool(name="small", bufs=2) as spool,
            tc.tile_pool(name="y", bufs=4) as ypool,
            tc.tile_pool(name="ps", bufs=4, space="PSUM") as pspool,
        ):
            # ---- persistent SBUF tensors ----
            xT_sb = cpool.tile([128, CT, N], bf16)            # 12KB/part
            wqk_sb = cpool.tile([128, CT, 2, CT, 128], bf16)  # 18KB
            wv_sb = cpool.tile([128, CT, C], bf16)            # 9KB
            wp_sb = cpool.tile([128, CT, C], bf16)            # 9KB
            pb_sb = cpool.tile([128, CT], f32)
            # v and Onorm^T as per-nt / per-kt tiles: exact dependency
            # granularity (a write to one chunk never serializes readers
            # of the others)
            v_sb = [cpool.tile([128, H, D + 1], bf16, name=f"v{nt}")
                    for nt in range(NT)]                      # 12.2KB
            on_sb = [cpool.tile([128, NCH, 512], bf16, name=f"on{kt}")
                     for kt in range(CT)]                     # 12KB

            # input DMA, three queues by criticality: gpsimd's software
            # DGE (own issue budget) carries xT; the two HWDGE queues
            # (sync/scalar, shared issue bandwidth) carry pair-0 q+k and
            # wv first, then the remaining wqk pairs, then wp.
            for kt in range(CT):
                r = kt * 128
                nc.gpsimd.dma_start(xT_sb[:, kt, :], xT_d[r:r + 128, :])
                nc.sync.dma_start(wqk_sb[:, kt, :, 0, :],
                                  wqk_d[r:r + 128, :, 0, :])
                nc.scalar.dma_start(wv_sb[:, kt, :], wv_d[r:r + 128, :])
            for kt in range(CT):
                r = kt * 128
                nc.sync.dma_start(wqk_sb[:, kt, :, 1:CT, :],
                                  wqk_d[r:r + 128, :, 1:CT, :])
            for kt in range(CT):
                r = kt * 128
                nc.scalar.dma_start(wp_sb[:, kt, :], wp_d[r:r + 128, :])
            nc.gpsimd.dma_start(pb_sb[:], pb_d[:])
            # ones column fused into v (softmax sums emerge as O^T row 64)
            for nt in range(NT):
                nc.vector.memset(v_sb[nt][:, :, D:D + 1], 1.0)

            def body():
                qks = {}      # (hp, role) -> SBUF tile; ("ps",hp,role) -> psum
                vps = {}

                def emit_qk_chunk(hp, role, lo, hi):
                    key = ("ps", hp, role)
                    if lo == 0:
                        qks[key] = pspool.tile([128, NCH, 512], f32, tag="ps",
                                               name=f"qk{hp}r{role}")
                    ps = qks[key]
                    for kt in range(lo, hi):
                        for nch in range(NCH):
                            nc.tensor.matmul(
                                ps[:, nch, :],
                                wqk_sb[:, kt, role, hp, :],
                                xT_sb[:, kt, nch * 512:(nch + 1) * 512],
                                start=(kt == 0), stop=(kt == CT - 1),
                            )
                    if hi == CT:
                        t = qkpool.tile([128, NCH, 512], bf16, tag="qk",
                                        name=f"qk{hp}r{role}sb")
                        nc.vector.tensor_copy(t[:], ps[:, :, :])
                        qks[(hp, role)] = t
                        del qks[key]

                def emit_v_chunk(nt, lo, hi):
                    if lo == 0:
                        vps[nt] = pspool.tile([128, 2, 8, 64], f32, tag="ps",
                                              name=f"v{nt}")
                    ps = vps[nt]
                    for kt in range(lo, hi):
                        for och in range(2):
                            nc.tensor.matmul(
                                ps[:, och, 0:6, :],
                                xT_sb[:, kt, nt * 128:(nt + 1) * 128],
                                wv_sb[:, kt, och * 384:(och + 1) * 384],
                                start=(kt == 0), stop=(kt == CT - 1),
                            )
                    if hi == CT:
                        for och in range(2):
                            nc.vector.tensor_copy(
                                v_sb[nt][:, och * 6:(och + 1) * 6, 0:D],
                                ps[:, och, 0:6, :],
                            )
                        del vps[nt]

                fillers = []

                def take_fillers(k):
                    for _ in range(min(k, len(fillers))):
                        fillers.pop(0)()

                def queue_qk_chunks(hp):
                    for role in (0, 1):
                        for kt in range(CT):
                            fillers.append(
                                lambda hp=hp, role=role, kt=kt:
                                emit_qk_chunk(hp, role, kt, kt + 1))

                def queue_v_chunks(nt):
                    fillers.append(lambda: emit_v_chunk(nt, 0, 3))
                    fillers.append(lambda: emit_v_chunk(nt, 3, CT))

                def norm(h, ps_o, tail=False):
                    """O^T[0:64]/sums(row64) -> on_sb. The DVE drain copy
                    frees the PSUM slot early; Pool does broadcast+mult."""
                    hp, odd = h // 2, h % 2
                    # reciprocal_approx_fast misreads PSUM@partition-64 on
                    # HW - stage the sums row through SBUF first
                    sm = spool.tile([1, NCH, 512], f32, tag="sm")
                    if tail:
                        nc.scalar.copy(sm[:], ps_o[64:65, :, :])
                    else:
                        nc.vector.tensor_copy(sm[:], ps_o[64:65, :, :])
                    rec = spool.tile([1, NCH, 512], f32, tag="rec")
                    nc.vector.reciprocal_approx_fast(rec[:], sm[:])
                    ou = spool.tile([64, NCH, 512], bf16, tag="ou")
                    if tail:
                        nc.scalar.copy(ou[:], ps_o[0:64, :, :])
                    else:
                        nc.vector.tensor_copy(ou[:], ps_o[0:64, :, :])
                    R = spool.tile([64, NCH, 512], f32, tag="R")
                    nc.gpsimd.partition_broadcast(R[:], rec[:])
                    # odd heads norm at group 7: their multiply goes to
                    # DVE so Pool's phase-boundary queue stays short
                    eng = nc.vector if (tail or odd) else nc.gpsimd
                    eng.tensor_tensor(
                        on_sb[hp][64 * odd:64 * odd + 64, :, :],
                        ou[:], R[:], op=mybir.AluOpType.mult,
                    )

                # ---- startup: eager qk(pair 0) pipelined with input DMA ----
                emit_qk_chunk(0, 0, 0, CT)
                emit_qk_chunk(0, 1, 0, CT)
                emit_v_chunk(0, 0, CT)
                emit_v_chunk(1, 0, CT)

                E_prev = None
                pj = {}
                stg = {}

                def stage_proj(otp):
                    t = ypool.tile([128, NCH, 512], bf16, tag="stg",
                                   name=f"stg{otp}")
                    nc.vector.tensor_copy(t[:], pj[otp][:, :, :])
                    stg[otp] = t
                    del pj[otp]

                def proj_chunk(otp, kts, lo=0, stop_kt=CT - 1):
                    if kts[0] == lo:
                        pj[otp] = pspool.tile([128, NCH, 512], f32, tag="ps",
                                              name=f"pj{otp}k{lo}")
                    for kt in kts:
                        for nch in range(NCH):
                            nc.tensor.matmul(
                                pj[otp][:, nch, :],
                                wp_sb[:, kt, otp * 128:(otp + 1) * 128],
                                on_sb[kt][:, nch, :],
                                start=(kt == lo), stop=(kt == stop_kt),
                            )

                def proj_fin(otp):
                    # per-nch epilogue so the Act copy and output DMA of
                    # nch0 overlap the tail's remaining PE work
                    yt = ypool.tile([128, NCH, 512], bf16, tag="yt")
                    for nch in range(NCH):
                        nc.scalar.activation(
                            yt[:, nch, :], pj[otp][:, nch, :],
                            mybir.ActivationFunctionType.Identity,
                            bias=pb_sb[:, otp:otp + 1],
                        )
                        # split across software/hardware DGE queues
                        eng = nc.gpsimd if nch == 0 else nc.sync
                        eng.dma_start(
                            out_d[otp * 128:(otp + 1) * 128,
                                  nch * 512:(nch + 1) * 512],
                            yt[:, nch, :])

                for hp in range(HP):
                    q_sb, k_sb = qks[(hp, 0)], qks[(hp, 1)]
                    E_ab = epool.tile([128, NT, NCH, 2, 512], bf16, tag="E",
                                      name=f"E{hp}")
                    if hp == 0:
                        # qk(1) first: its SBUF copy gates phase 1's S MMs
                        queue_qk_chunks(1)
                        for nt in range(2, 6):
                            queue_v_chunks(nt)
                    elif hp == 1:
                        # v(6/7) first: O(pair 0) group 3 reads them mid-phase
                        for nt in range(6, 8):
                            queue_v_chunks(nt)
                        queue_qk_chunks(2)
                    elif hp < HP - 1:
                        queue_qk_chunks(hp + 1)
                    else:
                        # phase 5: proj otp-0/1/2 partials over ready kt 0-3,
                        # staged to SBUF via DVE (transient PSUM slot each)
                        for otp in (0, 1):
                            for kt in range(3):
                                fillers.append(
                                    lambda otp=otp, kt=kt:
                                    proj_chunk(otp, [kt], stop_kt=3))
                            fillers.append(
                                lambda otp=otp:
                                proj_chunk(otp, [3], stop_kt=3))
                            fillers.append(
                                lambda otp=otp: stage_proj(otp))

                    # O-prev sub-tiles per group: groups 1-3 head a',
                    # groups 5-7 head b'; groups 0/4 are PSUM drain windows.
                    osub = {1: (0, 3), 2: (3, 6), 3: (6, 8),
                            5: (0, 3), 6: (3, 6), 7: (6, 8)}
                    ps_o = None
                    for mt in range(NT):
                        c0, c1 = mt // 4, (mt % 4) * 128
                        for nch in range(NCH):
                            ps_s = pspool.tile([128, 2, 512], f32, tag="ps",
                                               name=f"s{hp}m{mt}n{nch}")
                            nc.tensor.matmul(
                                ps_s[:, 0, :],
                                k_sb[0:64, c0, c1:c1 + 128],
                                q_sb[0:64, nch, :], start=True, stop=True,
                            )
                            nc.tensor.matmul(
                                ps_s[:, 1, :],
                                k_sb[64:128, c0, c1:c1 + 128],
                                q_sb[64:128, nch, :], start=True, stop=True,
                            )
                            nc.scalar.activation(
                                E_ab[:, mt, nch, :, :], ps_s[:, :, :],
                                mybir.ActivationFunctionType.Exp, scale=SCALE,
                            )
                        if E_prev is not None and mt in osub:
                            h_prev = 2 * (hp - 1) + (0 if mt <= 3 else 1)
                            ab = h_prev % 2
                            lo, hi = osub[mt]
                            if lo == 0:
                                ps_o = pspool.tile([65, NCH, 512], f32,
                                                   tag="ps", name=f"o{h_prev}")
                            for sub in range(lo, hi):
                                for nch in range(NCH):
                                    nc.tensor.matmul(
                                        ps_o[:, nch, :],
                                        v_sb[sub][:, h_prev, :],
                                        E_prev[:, sub, nch, ab, :],
                                        start=(sub == 0), stop=(sub == NT - 1),
                                    )
                            if hi == NT:
                                norm(h_prev, ps_o)
                        # front-load pops so next pair's qk copy lands
                        # mid-phase, but keep late groups fed
                        nfill = 3 if (mt < 2 or len(fillers) > 16) else 2
                        take_fillers(nfill)
                    take_fillers(len(fillers))
                    E_prev = E_ab

                # ---- tail: O(pair 5) + projections, dense on PE ----
                ha, hb = 2 * (HP - 1), 2 * (HP - 1) + 1

                def emit_o_dense(h):
                    ps = pspool.tile([65, NCH, 512], f32, tag="ps",
                                     name=f"ot{h}")
                    ab = h % 2
                    for sub in range(NT):
                        for nch in range(NCH):
                            nc.tensor.matmul(
                                ps[:, nch, :], v_sb[sub][:, h, :],
                                E_prev[:, sub, nch, ab, :],
                                start=(sub == 0), stop=(sub == NT - 1),
                            )
                    return ps

                def proj_fin_staged(otp):
                    # (kt4+kt5 psum + bias) + staged kt0-3 partial, on DVE
                    yt = ypool.tile([128, NCH, 512], bf16, tag="yt")
                    nc.vector.scalar_tensor_tensor(
                        yt[:], pj[otp][:, :, :], pb_sb[:, otp:otp + 1],
                        stg[otp][:],
                        op0=mybir.AluOpType.add, op1=mybir.AluOpType.add,
                    )
                    for nch in range(NCH):
                        eng = nc.gpsimd if nch == 0 else nc.sync
                        eng.dma_start(
                            out_d[otp * 128:(otp + 1) * 128,
                                  nch * 512:(nch + 1) * 512],
                            yt[:, nch, :])

                ps_oa = emit_o_dense(ha)
                norm(ha, ps_oa, tail=True)
                ps_ob = emit_o_dense(hb)
                # overlap norm(11) chain with proj partial blocks
                proj_chunk(2, range(0, 5))
                norm(hb, ps_ob, tail=True)
                proj_chunk(3, range(0, 5))
                proj_chunk(0, [4, 5], lo=4)
                proj_fin_staged(0)
                proj_chunk(1, [4, 5], lo=4)
                proj_fin_staged(1)
                proj_chunk(2, [5])
                proj_fin(2)
                proj_chunk(3, [5])
                proj_fin(3)
                for otp in range(4, CT):
                    proj_chunk(otp, range(0, CT))
                    proj_fin(otp)

            if loop_r is not None:
                with tc.For_i(0, loop_r):
                    body()
            else:
                body()

    nc.compile()
    return nc


def _get_nc():
    if "nc" not in _CACHE:
        _CACHE["nc"] = _build_nc()
    return _CACHE["nc"]


def kernel(x, qkv_w, proj_w, proj_b):
    from concourse.bass_utils import run_bass_kernel_spmd

    nc = _get_nc()
    bf = ml_dtypes.bfloat16
    wqk = np.ascontiguousarray(qkv_w[:2 * C].T).astype(bf).reshape(C, 2, CT, 128)
    wv = np.ascontiguousarray(qkv_w[2 * C:].T).astype(bf)
    wp = np.ascontiguousarray(proj_w.T).astype(bf)
    pb = np.ascontiguousarray(proj_b.reshape(CT, 128).T).astype(np.float32)
    in_maps = []
    for i in range(B):
        in_maps.append({
            "xT": np.ascontiguousarray(x[i].T).astype(bf),
            "wqk": wqk, "wv": wv, "wp": wp, "pb": pb,
        })
    res = run_bass_kernel_spmd(nc, in_maps, core_ids=list(range(B)))
    out = np.stack([res.results[i]["out"].astype(np.float32).T for i in range(B)])
    return np.ascontiguousarray(out)
